# revision 17
# baseline (speedup 1.0000x reference)
"""DINet retrieval-knn kernel for 8 trn2 NeuronCores.

Math (see reference): for each query patch q (3x3xC neighborhood of Q),
find k* = argmax_k cos(K_patch_k, Q_patch_q) over all 4096 key patches,
output S = max cosine value, T = fold(V_patch_gather(k*)) / 9.

Device strategy (per sharding hint): data-parallel over batch B (=2),
sequence-parallel over Q columns (4 shards of 1024) -> 8 cores. Each core
computes its full [Lk=4096, Lq=1024] correlation block with the tensor
engine (contraction C*9=576 in fp32), and a fused
copy+max (tensor_tensor_reduce) plus max_index pass gives max/argmax over
the full K axis per query. Host does layout prep (unfold, l2-normalize)
and the final V-gather + fold.
"""

import sys

import numpy as np

for _p in ("/opt/trn_rl_repo", "/root/.axon_site/_ro/trn_rl_repo"):
    if _p not in sys.path:
        sys.path.append(_p)

import concourse.bass as bass
import concourse.mybir as mybir
from concourse import bacc, bass_utils
from concourse.tile import TileContext

B, C, H, W = 2, 64, 64, 64
L = H * W            # 4096
C9 = C * 9           # 576
NSHARD = 4           # Q-column shards per batch
LQ = L // NSHARD     # 1024 query columns per core
NCORES = 8
NQB = LQ // 128      # 8 query blocks of 128
NKT = L // 512       # 8 key column tiles of 512
# contraction chunks over C9=576: rows (start, size)
CHUNKS = [(0, 128), (128, 128), (256, 128), (384, 128), (512, 64)]

EPS = 1e-12

_BASS_CACHE = {}


def _build_bass():
    f32 = mybir.dt.float32
    bf16 = mybir.dt.bfloat16  # full-rate PE + FWL weight loads + half DMA
    u32 = mybir.dt.uint32
    # Bacc (not plain Bass): its compile() runs move_matmul_waits_to_ldweights
    # + generate_event_semaphores, which split multi-wait instructions that
    # walrus otherwise rejects ("Too many sync wait commands")
    nc = bacc.Bacc("TRN2")

    # chunk-interleaved layouts (host packs): *A tensors hold contraction
    # chunks t=0..3 side by side so one DMA feeds a full k-slice; *B holds
    # the 64-row tail chunk (t=4)
    knA = nc.dram_tensor("knA", [128, NKT * 4 * 512], bf16, kind="ExternalInput")
    knB = nc.dram_tensor("knB", [64, L], bf16, kind="ExternalInput")
    qnA = nc.dram_tensor("qnA", [128, 4 * LQ], bf16, kind="ExternalInput")
    qnB = nc.dram_tensor("qnB", [64, LQ], bf16, kind="ExternalInput")
    # top-8 candidate indices per (query column, 2048-wide k-half); host
    # re-scores the 16 candidates exactly in fp32
    rarg = nc.dram_tensor("rarg", [128, NQB * 16], u32, kind="ExternalOutput")

    with TileContext(nc) as tc:
        with (
            tc.tile_pool(name="kpool", bufs=3) as kpool,
            tc.tile_pool(name="qpool", bufs=1) as qpool,
            tc.tile_pool(name="rpool", bufs=1) as rpool,
            tc.tile_pool(name="ppool", bufs=8, space="PSUM") as ppool,
            tc.tile_pool(name="mpool", bufs=4) as mpool,
            tc.tile_pool(name="opool", bufs=1) as opool,
        ):
            qtA = [
                qpool.tile([128, LQ], bf16, name=f"qtA{t}", tag=f"qtA{t}")
                for t in range(4)
            ]
            qtB = qpool.tile([64, LQ], bf16, name="qtB", tag="qtB")
            for t in range(4):
                nc.sync.dma_start(out=qtA[t], in_=qnA[:, t * LQ : (t + 1) * LQ])
            nc.sync.dma_start(out=qtB, in_=qnB[:, :])

            rt = [
                rpool.tile([128, L], bf16, name=f"rt{qb}", tag=f"rt{qb}")
                for qb in range(NQB)
            ]
            outi = opool.tile([128, NQB * 16], u32, name="outi", tag="outi")

            for n in range(NKT):
                ktA = kpool.tile([128, 4 * 512], bf16, name="ktA", tag="ktA")
                nc.sync.dma_start(out=ktA, in_=knA[:, n * 2048 : (n + 1) * 2048])
                ktB = kpool.tile([64, 512], bf16, name="ktB", tag="ktB")
                nc.sync.dma_start(out=ktB, in_=knB[:, n * 512 : (n + 1) * 512])
                for qb in range(NQB):
                    ps = ppool.tile([128, 512], f32, name="ps", tag="ps")
                    for t in range(4):
                        nc.tensor.matmul(
                            ps,
                            lhsT=qtA[t][:, qb * 128 : (qb + 1) * 128],
                            rhs=ktA[:, t * 512 : (t + 1) * 512],
                            start=(t == 0),
                            stop=False,
                        )
                    nc.tensor.matmul(
                        ps,
                        lhsT=qtB[:, qb * 128 : (qb + 1) * 128],
                        rhs=ktB,
                        start=False,
                        stop=True,
                    )
                    # downcast to bf16: halves the DVE scan cost (2x mode)
                    nc.scalar.copy(out=rt[qb][:, n * 512 : (n + 1) * 512], in_=ps)
                    if n % 4 == 3:
                        kh = n // 4
                        half = rt[qb][:, kh * 2048 : (kh + 1) * 2048]
                        mx8 = mpool.tile([128, 8], bf16, name="mx8", tag="mx8")
                        nc.vector.max(out=mx8, in_=half)
                        idx8 = mpool.tile([128, 8], u32, name="idx8", tag="idx8")
                        nc.vector.max_index(out=idx8, in_max=mx8, in_values=half)
                        nc.gpsimd.tensor_copy(
                            out=outi[
                                :, (qb * 2 + kh) * 8 : (qb * 2 + kh + 1) * 8
                            ],
                            in_=idx8,
                        )

            nc.sync.dma_start(out=rarg[:, :], in_=outi)
    if not nc.is_finalized():
        nc.finalize()
    return nc


def _unfold_ij(x):
    """[B,C,H,W] -> [B, 9*C, H*W] with row = ij*C + c (ij-major order)."""
    b, c, h, w = x.shape
    xp = np.pad(x, ((0, 0), (0, 0), (1, 1), (1, 1)))
    blocks = [
        xp[:, :, i : i + h, j : j + w].reshape(b, c, h * w)
        for i in range(3)
        for j in range(3)
    ]
    return np.concatenate(blocks, axis=1)


def _unfold_torch(x):
    """[B,C,H,W] -> [B, C*9, H*W] in torch F.unfold order (c-major)."""
    b, c, h, w = x.shape
    xp = np.pad(x, ((0, 0), (0, 0), (1, 1), (1, 1)))
    patches = np.stack(
        [xp[:, :, i : i + h, j : j + w] for i in range(3) for j in range(3)],
        axis=2,
    )
    return patches.reshape(b, c * 9, h * w)


def _fold_torch(u, h, w):
    """Inverse layout of _unfold_torch: sum overlapping patches."""
    b, ck, l = u.shape
    c = ck // 9
    p = u.reshape(b, c, 3, 3, h, w)
    out = np.zeros((b, c, h + 2, w + 2), u.dtype)
    for i in range(3):
        for j in range(3):
            out[:, :, i : i + h, j : j + w] += p[:, :, i, j]
    return out[:, :, 1 : 1 + h, 1 : 1 + w]


def _l2n_cols(x):
    """Normalize columns of [B, C9, L] (fp32, eps as in reference)."""
    n = np.sqrt(np.sum(x * x, axis=1, keepdims=True, dtype=np.float32))
    return x / np.maximum(n, EPS)


def _pack_A(x):
    """[512, M] -> chunk-interleaved [128, (M/512)*4*512]: out[p, n, t, c] =
    x[t*128 + p, n*512 + c]."""
    m = x.shape[1]
    nn = m // 512
    return np.ascontiguousarray(
        x.reshape(4, 128, nn, 512).transpose(1, 2, 0, 3).reshape(128, nn * 4 * 512)
    )


def _run_device(Kn, Qn, trace=False, trace_cores=None):
    import ml_dtypes

    key = "nc"
    if key not in _BASS_CACHE:
        _BASS_CACHE[key] = _build_bass()
    nc = _BASS_CACHE[key]
    bf = ml_dtypes.bfloat16
    in_maps = []
    for ci in range(NCORES):
        b, s = divmod(ci, NSHARD)
        kb = Kn[b].astype(bf)
        qb = Qn[b][:, s * LQ : (s + 1) * LQ].astype(bf)
        in_maps.append(
            {
                "knA": _pack_A(kb[:512]),
                "knB": np.ascontiguousarray(kb[512:]),
                # qnA packs [512, LQ] as [128, 4*LQ] with chunk t major
                "qnA": np.ascontiguousarray(
                    qb[:512].reshape(4, 128, LQ).transpose(1, 0, 2).reshape(128, 4 * LQ)
                ),
                "qnB": np.ascontiguousarray(qb[512:]),
            }
        )
    res = bass_utils.run_bass_kernel_spmd(
        nc,
        in_maps,
        core_ids=list(range(NCORES)),
        trace=trace,
        trace_cores=trace_cores,
    )
    return res


def kernel(V, K, Q, _trace=False, _trace_cores=None, _return_results=False):
    V = np.asarray(V, dtype=np.float32)
    K = np.asarray(K, dtype=np.float32)
    Q = np.asarray(Q, dtype=np.float32)

    Kn = _l2n_cols(_unfold_ij(K))
    Qn = _l2n_cols(_unfold_ij(Q))

    res = _run_device(Kn, Qn, trace=_trace, trace_cores=_trace_cores)

    # device returns top-8 candidate k per (query, 2048-wide k-half) under
    # bf16 matmul scores; re-score the 16 candidates exactly in fp32
    cand = np.empty((B, L, 16), np.int64)
    for ci in range(NCORES):
        b, s = divmod(ci, NSHARD)
        out = np.asarray(res.results[ci]["rarg"]).astype(np.int64)
        # out[p, (qb*2+kh)*8+j] -> local q index qb*128 + p, k = kh*2048 + idx
        c = out.reshape(128, NQB, 2, 8)
        c = np.clip(c, 0, 2047) + np.arange(2)[None, None, :, None] * 2048
        cand[b, s * LQ : (s + 1) * LQ] = (
            c.reshape(128, NQB, 16).transpose(1, 0, 2).reshape(LQ, 16)
        )

    rstar = np.empty((B, L), np.float32)
    rarg = np.empty((B, L), np.int64)
    for b in range(B):
        kc = Kn[b][:, cand[b].reshape(-1)].reshape(C9, L, 16)
        scores = np.einsum("cqj,cq->qj", kc, Qn[b], dtype=np.float64)
        maxv = scores.max(axis=1, keepdims=True)
        kmask = np.where(scores == maxv, cand[b], 1 << 40)
        rarg[b] = kmask.min(axis=1)  # first occurrence on ties, like argmax
        rstar[b] = maxv[:, 0].astype(np.float32)

    V_unf = _unfold_torch(V)
    T_unf = np.take_along_axis(V_unf, rarg[:, None, :], axis=2)
    T = (_fold_torch(T_unf, H, W) / 9.0).astype(np.float32)
    S = rstar.reshape(B, 1, H, W)

    if _return_results:
        return (S, T), res
    return (S, T)


# revision 18
# speedup vs baseline: 1.1945x; 1.1945x over previous
"""DINet retrieval-knn kernel for 8 trn2 NeuronCores.

Math (see reference): for each query patch q (3x3xC neighborhood of Q),
find k* = argmax_k cos(K_patch_k, Q_patch_q) over all 4096 key patches,
output S = max cosine value, T = fold(V_patch_gather(k*)) / 9.

Device strategy (per sharding hint): data-parallel over batch B (=2),
sequence-parallel over Q columns (4 shards of 1024) -> 8 cores. Each core
computes its full [Lk=4096, Lq=1024] correlation block with the tensor
engine (contraction C*9=576 in fp32), and a fused
copy+max (tensor_tensor_reduce) plus max_index pass gives max/argmax over
the full K axis per query. Host does layout prep (unfold, l2-normalize)
and the final V-gather + fold.
"""

import sys

import numpy as np

for _p in ("/opt/trn_rl_repo", "/root/.axon_site/_ro/trn_rl_repo"):
    if _p not in sys.path:
        sys.path.append(_p)

import concourse.bass as bass
import concourse.mybir as mybir
from concourse import bacc, bass_utils
from concourse.tile import TileContext

B, C, H, W = 2, 64, 64, 64
L = H * W            # 4096
C9 = C * 9           # 576
NSHARD = 4           # Q-column shards per batch
LQ = L // NSHARD     # 1024 query columns per core
NCORES = 8
NQB = LQ // 128      # 8 query blocks of 128
NKT = L // 512       # 8 key column tiles of 512
# contraction chunks over C9=576: rows (start, size)
CHUNKS = [(0, 128), (128, 128), (256, 128), (384, 128), (512, 64)]

EPS = 1e-12

_BASS_CACHE = {}


def _build_bass():
    f32 = mybir.dt.float32
    bf16 = mybir.dt.bfloat16  # full-rate PE + FWL weight loads + half DMA
    u32 = mybir.dt.uint32
    # Bacc (not plain Bass): its compile() runs move_matmul_waits_to_ldweights
    # + generate_event_semaphores, which split multi-wait instructions that
    # walrus otherwise rejects ("Too many sync wait commands")
    nc = bacc.Bacc("TRN2")

    # chunk-interleaved layouts (host packs): *A tensors hold contraction
    # chunks t=0..3, *B the 64-row tail chunk (t=4).
    # knA[p, kh, t, c] = kn[t*128+p, kh*2048+c]
    knA = nc.dram_tensor("knA", [128, 2 * 4 * 2048], bf16, kind="ExternalInput")
    knB = nc.dram_tensor("knB", [64, L], bf16, kind="ExternalInput")
    # qnA[p, t, q] = qn[t*128+p, q]
    qnA = nc.dram_tensor("qnA", [128, 4 * LQ], bf16, kind="ExternalInput")
    qnB = nc.dram_tensor("qnB", [64, LQ], bf16, kind="ExternalInput")
    # top-8 candidate indices per (query column, 2048-wide k-half); host
    # re-scores the 16 candidates exactly in fp32
    rarg = nc.dram_tensor("rarg", [128, NQB * 16], u32, kind="ExternalOutput")

    with TileContext(nc) as tc:
        with (
            tc.tile_pool(name="kpool", bufs=1) as kpool,
            tc.tile_pool(name="qpool", bufs=1) as qpool,
            tc.tile_pool(name="rpool", bufs=3) as rpool,
            tc.tile_pool(name="ppool", bufs=8, space="PSUM") as ppool,
            tc.tile_pool(name="mpool", bufs=4) as mpool,
            tc.tile_pool(name="opool", bufs=1) as opool,
        ):
            qtA = [
                qpool.tile([128, LQ], bf16, name=f"qtA{t}", tag=f"qtA{t}")
                for t in range(4)
            ]
            qtB = qpool.tile([64, LQ], bf16, name="qtB", tag="qtB")
            ktA = [
                [
                    kpool.tile([128, 2048], bf16, name=f"ktA{kh}_{t}", tag=f"ktA{kh}_{t}")
                    for t in range(4)
                ]
                for kh in range(2)
            ]
            ktB = [
                kpool.tile([64, 2048], bf16, name=f"ktB{kh}", tag=f"ktB{kh}")
                for kh in range(2)
            ]
            # DMA issue order ~ consumption order; 15 input DMAs total
            nc.sync.dma_start(out=qtA[0], in_=qnA[:, 0:LQ])
            for t in range(4):
                nc.sync.dma_start(
                    out=ktA[0][t], in_=knA[:, t * 2048 : (t + 1) * 2048]
                )
            for t in range(1, 4):
                nc.sync.dma_start(out=qtA[t], in_=qnA[:, t * LQ : (t + 1) * LQ])
            nc.sync.dma_start(out=qtB, in_=qnB[:, :])
            nc.sync.dma_start(out=ktB[0], in_=knB[:, 0:2048])
            for t in range(4):
                nc.sync.dma_start(
                    out=ktA[1][t], in_=knA[:, (4 + t) * 2048 : (5 + t) * 2048]
                )
            nc.sync.dma_start(out=ktB[1], in_=knB[:, 2048:4096])

            outi = opool.tile([128, NQB * 16], u32, name="outi", tag="outi")

            for qb in range(NQB):
                for kh in range(2):
                    rt = rpool.tile([128, 2048], bf16, name="rt", tag="rt")
                    for n in range(4):
                        ps = ppool.tile([128, 512], f32, name="ps", tag="ps")
                        for t in range(4):
                            nc.tensor.matmul(
                                ps,
                                lhsT=qtA[t][:, qb * 128 : (qb + 1) * 128],
                                rhs=ktA[kh][t][:, n * 512 : (n + 1) * 512],
                                start=(t == 0),
                                stop=False,
                            )
                        nc.tensor.matmul(
                            ps,
                            lhsT=qtB[:, qb * 128 : (qb + 1) * 128],
                            rhs=ktB[kh][:, n * 512 : (n + 1) * 512],
                            start=False,
                            stop=True,
                        )
                        # downcast to bf16: halves the DVE scan cost (2x mode)
                        nc.scalar.copy(
                            out=rt[:, n * 512 : (n + 1) * 512], in_=ps
                        )
                    mx8 = mpool.tile([128, 8], bf16, name="mx8", tag="mx8")
                    nc.vector.max(out=mx8, in_=rt)
                    idx8 = mpool.tile([128, 8], u32, name="idx8", tag="idx8")
                    nc.vector.max_index(out=idx8, in_max=mx8, in_values=rt)
                    nc.gpsimd.tensor_copy(
                        out=outi[:, (qb * 2 + kh) * 8 : (qb * 2 + kh + 1) * 8],
                        in_=idx8,
                    )

            nc.sync.dma_start(out=rarg[:, :], in_=outi)
    if not nc.is_finalized():
        nc.finalize()
    return nc


def _unfold_ij(x):
    """[B,C,H,W] -> [B, 9*C, H*W] with row = ij*C + c (ij-major order)."""
    b, c, h, w = x.shape
    xp = np.pad(x, ((0, 0), (0, 0), (1, 1), (1, 1)))
    blocks = [
        xp[:, :, i : i + h, j : j + w].reshape(b, c, h * w)
        for i in range(3)
        for j in range(3)
    ]
    return np.concatenate(blocks, axis=1)


def _unfold_torch(x):
    """[B,C,H,W] -> [B, C*9, H*W] in torch F.unfold order (c-major)."""
    b, c, h, w = x.shape
    xp = np.pad(x, ((0, 0), (0, 0), (1, 1), (1, 1)))
    patches = np.stack(
        [xp[:, :, i : i + h, j : j + w] for i in range(3) for j in range(3)],
        axis=2,
    )
    return patches.reshape(b, c * 9, h * w)


def _fold_torch(u, h, w):
    """Inverse layout of _unfold_torch: sum overlapping patches."""
    b, ck, l = u.shape
    c = ck // 9
    p = u.reshape(b, c, 3, 3, h, w)
    out = np.zeros((b, c, h + 2, w + 2), u.dtype)
    for i in range(3):
        for j in range(3):
            out[:, :, i : i + h, j : j + w] += p[:, :, i, j]
    return out[:, :, 1 : 1 + h, 1 : 1 + w]


def _l2n_cols(x):
    """Normalize columns of [B, C9, L] (fp32, eps as in reference)."""
    n = np.sqrt(np.sum(x * x, axis=1, keepdims=True, dtype=np.float32))
    return x / np.maximum(n, EPS)


def _pack_A(x):
    """[512, 4096] -> [128, 2*4*2048]: out[p, kh, t, c] = x[t*128+p, kh*2048+c]."""
    return np.ascontiguousarray(
        x.reshape(4, 128, 2, 2048).transpose(1, 2, 0, 3).reshape(128, 2 * 4 * 2048)
    )


def _run_device(Kn, Qn, trace=False, trace_cores=None):
    import ml_dtypes

    key = "nc"
    if key not in _BASS_CACHE:
        _BASS_CACHE[key] = _build_bass()
    nc = _BASS_CACHE[key]
    bf = ml_dtypes.bfloat16
    in_maps = []
    for ci in range(NCORES):
        b, s = divmod(ci, NSHARD)
        kb = Kn[b].astype(bf)
        qb = Qn[b][:, s * LQ : (s + 1) * LQ].astype(bf)
        in_maps.append(
            {
                "knA": _pack_A(kb[:512]),
                "knB": np.ascontiguousarray(kb[512:]),
                # qnA packs [512, LQ] as [128, 4*LQ] with chunk t major
                "qnA": np.ascontiguousarray(
                    qb[:512].reshape(4, 128, LQ).transpose(1, 0, 2).reshape(128, 4 * LQ)
                ),
                "qnB": np.ascontiguousarray(qb[512:]),
            }
        )
    res = bass_utils.run_bass_kernel_spmd(
        nc,
        in_maps,
        core_ids=list(range(NCORES)),
        trace=trace,
        trace_cores=trace_cores,
    )
    return res


def kernel(V, K, Q, _trace=False, _trace_cores=None, _return_results=False):
    V = np.asarray(V, dtype=np.float32)
    K = np.asarray(K, dtype=np.float32)
    Q = np.asarray(Q, dtype=np.float32)

    Kn = _l2n_cols(_unfold_ij(K))
    Qn = _l2n_cols(_unfold_ij(Q))

    res = _run_device(Kn, Qn, trace=_trace, trace_cores=_trace_cores)

    # device returns top-8 candidate k per (query, 2048-wide k-half) under
    # bf16 matmul scores; re-score the 16 candidates exactly in fp32
    cand = np.empty((B, L, 16), np.int64)
    for ci in range(NCORES):
        b, s = divmod(ci, NSHARD)
        out = np.asarray(res.results[ci]["rarg"]).astype(np.int64)
        # out[p, (qb*2+kh)*8+j] -> local q index qb*128 + p, k = kh*2048 + idx
        c = out.reshape(128, NQB, 2, 8)
        c = np.clip(c, 0, 2047) + np.arange(2)[None, None, :, None] * 2048
        cand[b, s * LQ : (s + 1) * LQ] = (
            c.reshape(128, NQB, 16).transpose(1, 0, 2).reshape(LQ, 16)
        )

    rstar = np.empty((B, L), np.float32)
    rarg = np.empty((B, L), np.int64)
    for b in range(B):
        kc = Kn[b][:, cand[b].reshape(-1)].reshape(C9, L, 16)
        scores = np.einsum("cqj,cq->qj", kc, Qn[b], dtype=np.float64)
        maxv = scores.max(axis=1, keepdims=True)
        kmask = np.where(scores == maxv, cand[b], 1 << 40)
        rarg[b] = kmask.min(axis=1)  # first occurrence on ties, like argmax
        rstar[b] = maxv[:, 0].astype(np.float32)

    V_unf = _unfold_torch(V)
    T_unf = np.take_along_axis(V_unf, rarg[:, None, :], axis=2)
    T = (_fold_torch(T_unf, H, W) / 9.0).astype(np.float32)
    S = rstar.reshape(B, 1, H, W)

    if _return_results:
        return (S, T), res
    return (S, T)


# revision 22
# speedup vs baseline: 1.2025x; 1.0067x over previous
"""DINet retrieval-knn kernel for 8 trn2 NeuronCores.

Math (see reference): for each query patch q (3x3xC neighborhood of Q),
find k* = argmax_k cos(K_patch_k, Q_patch_q) over all 4096 key patches,
output S = max cosine value, T = fold(V_patch_gather(k*)) / 9.

Device strategy (per sharding hint): data-parallel over batch B (=2),
sequence-parallel over Q columns (4 shards of 1024) -> 8 cores. Each core
computes its full [Lk=4096, Lq=1024] correlation block with the tensor
engine (contraction C*9=576 in fp32), and a fused
copy+max (tensor_tensor_reduce) plus max_index pass gives max/argmax over
the full K axis per query. Host does layout prep (unfold, l2-normalize)
and the final V-gather + fold.
"""

import sys

import numpy as np

for _p in ("/opt/trn_rl_repo", "/root/.axon_site/_ro/trn_rl_repo"):
    if _p not in sys.path:
        sys.path.append(_p)

import concourse.bass as bass
import concourse.mybir as mybir
from concourse import bacc, bass_utils
from concourse.tile import TileContext

B, C, H, W = 2, 64, 64, 64
L = H * W            # 4096
C9 = C * 9           # 576
NSHARD = 4           # Q-column shards per batch
LQ = L // NSHARD     # 1024 query columns per core
NCORES = 8
NQB = LQ // 128      # 8 query blocks of 128
NKT = L // 512       # 8 key column tiles of 512
# contraction chunks over C9=576: rows (start, size)
CHUNKS = [(0, 128), (128, 128), (256, 128), (384, 128), (512, 64)]

EPS = 1e-12

_BASS_CACHE = {}


def _build_bass():
    f32 = mybir.dt.float32
    bf16 = mybir.dt.bfloat16  # full-rate PE + FWL weight loads + half DMA
    u32 = mybir.dt.uint32
    # Bacc (not plain Bass): its compile() runs move_matmul_waits_to_ldweights
    # + generate_event_semaphores, which split multi-wait instructions that
    # walrus otherwise rejects ("Too many sync wait commands")
    nc = bacc.Bacc("TRN2")

    kn = nc.dram_tensor("kn", [C9, L], bf16, kind="ExternalInput")
    qn = nc.dram_tensor("qn", [C9, LQ], bf16, kind="ExternalInput")
    # top-8 candidate indices per (query column, 2048-wide k-half); the
    # last block reports two 1024-wide lists (tail latency), so one extra
    # 8-slot group; host re-scores all candidates exactly in fp32
    rarg = nc.dram_tensor("rarg", [128, (NQB * 2 + 1) * 8], u32, kind="ExternalOutput")

    with TileContext(nc) as tc:
        with (
            tc.tile_pool(name="kpool", bufs=1) as kpool,
            tc.tile_pool(name="qpool", bufs=1) as qpool,
            tc.tile_pool(name="rpool", bufs=3) as rpool,
            tc.tile_pool(name="ppool", bufs=4, space="PSUM") as ppool,
            tc.tile_pool(name="mpool", bufs=4) as mpool,
            tc.tile_pool(name="opool", bufs=1) as opool,
        ):
            qt = [
                qpool.tile([p, LQ], bf16, name=f"qt{t}", tag=f"qt{t}")
                for t, (r0, p) in enumerate(CHUNKS)
            ]
            kt = [
                [
                    kpool.tile([p, 2048], bf16, name=f"kt{t}_{kh}", tag=f"kt{t}_{kh}")
                    for kh in range(2)
                ]
                for t, (r0, p) in enumerate(CHUNKS)
            ]
            # DMA issue order ~ consumption order
            for t, (r0, p) in enumerate(CHUNKS):
                nc.sync.dma_start(out=qt[t], in_=qn[r0 : r0 + p, :])
            for kh in range(2):
                for t, (r0, p) in enumerate(CHUNKS):
                    nc.sync.dma_start(
                        out=kt[t][kh], in_=kn[r0 : r0 + p, kh * 2048 : (kh + 1) * 2048]
                    )

            outi = opool.tile([128, (NQB * 2 + 1) * 8], u32, name="outi", tag="outi")

            for qb in range(NQB):
                for kh in range(2):
                    rt = rpool.tile([128, 2048], bf16, name="rt", tag="rt")
                    # two 2-bank psum groups per block: the first SBUF copy
                    # happens after half the K data, shortening the prologue
                    # and the tail
                    for g in range(2):
                        ps = ppool.tile([128, 1024], f32, name="ps", tag="ps")
                        for t in range(5):
                            for n in range(2):
                                nc.tensor.matmul(
                                    ps[:, n * 512 : (n + 1) * 512],
                                    lhsT=qt[t][:, qb * 128 : (qb + 1) * 128],
                                    rhs=kt[t][kh][
                                        :, (g * 2 + n) * 512 : (g * 2 + n + 1) * 512
                                    ],
                                    start=(t == 0),
                                    stop=(t == 4),
                                )
                        # downcast to bf16: halves the DVE scan cost (2x mode)
                        nc.scalar.copy(
                            out=rt[:, g * 1024 : (g + 1) * 1024], in_=ps
                        )
                    last = qb == NQB - 1 and kh == 1
                    scans = ((0, 1024), (1024, 1024)) if last else ((0, 2048),)
                    for si, (c0, w) in enumerate(scans):
                        mx8 = mpool.tile([128, 8], bf16, name="mx8", tag="mx8")
                        nc.vector.max(out=mx8, in_=rt[:, c0 : c0 + w])
                        idx8 = mpool.tile([128, 8], u32, name="idx8", tag="idx8")
                        nc.vector.max_index(
                            out=idx8, in_max=mx8, in_values=rt[:, c0 : c0 + w]
                        )
                        dst0 = (qb * 2 + kh + si) * 8
                        nc.gpsimd.tensor_copy(
                            out=outi[:, dst0 : dst0 + 8], in_=idx8
                        )

            nc.sync.dma_start(out=rarg[:, :], in_=outi)
    if not nc.is_finalized():
        nc.finalize()
    return nc


def _unfold_ij(x):
    """[B,C,H,W] -> [B, 9*C, H*W] with row = ij*C + c (ij-major order)."""
    b, c, h, w = x.shape
    xp = np.pad(x, ((0, 0), (0, 0), (1, 1), (1, 1)))
    blocks = [
        xp[:, :, i : i + h, j : j + w].reshape(b, c, h * w)
        for i in range(3)
        for j in range(3)
    ]
    return np.concatenate(blocks, axis=1)


def _unfold_torch(x):
    """[B,C,H,W] -> [B, C*9, H*W] in torch F.unfold order (c-major)."""
    b, c, h, w = x.shape
    xp = np.pad(x, ((0, 0), (0, 0), (1, 1), (1, 1)))
    patches = np.stack(
        [xp[:, :, i : i + h, j : j + w] for i in range(3) for j in range(3)],
        axis=2,
    )
    return patches.reshape(b, c * 9, h * w)


def _fold_torch(u, h, w):
    """Inverse layout of _unfold_torch: sum overlapping patches."""
    b, ck, l = u.shape
    c = ck // 9
    p = u.reshape(b, c, 3, 3, h, w)
    out = np.zeros((b, c, h + 2, w + 2), u.dtype)
    for i in range(3):
        for j in range(3):
            out[:, :, i : i + h, j : j + w] += p[:, :, i, j]
    return out[:, :, 1 : 1 + h, 1 : 1 + w]


def _l2n_cols(x):
    """Normalize columns of [B, C9, L] (fp32, eps as in reference)."""
    n = np.sqrt(np.sum(x * x, axis=1, keepdims=True, dtype=np.float32))
    return x / np.maximum(n, EPS)


def _run_device(Kn, Qn, trace=False, trace_cores=None):
    import ml_dtypes

    key = "nc"
    if key not in _BASS_CACHE:
        _BASS_CACHE[key] = _build_bass()
    nc = _BASS_CACHE[key]
    bf = ml_dtypes.bfloat16
    in_maps = []
    for ci in range(NCORES):
        b, s = divmod(ci, NSHARD)
        in_maps.append(
            {
                "kn": np.ascontiguousarray(Kn[b].astype(bf)),
                "qn": np.ascontiguousarray(Qn[b][:, s * LQ : (s + 1) * LQ].astype(bf)),
            }
        )
    res = bass_utils.run_bass_kernel_spmd(
        nc,
        in_maps,
        core_ids=list(range(NCORES)),
        trace=trace,
        trace_cores=trace_cores,
    )
    return res


def kernel(V, K, Q, _trace=False, _trace_cores=None, _return_results=False):
    V = np.asarray(V, dtype=np.float32)
    K = np.asarray(K, dtype=np.float32)
    Q = np.asarray(Q, dtype=np.float32)

    Kn = _l2n_cols(_unfold_ij(K))
    Qn = _l2n_cols(_unfold_ij(Q))

    res = _run_device(Kn, Qn, trace=_trace, trace_cores=_trace_cores)

    # device returns top-8 candidate k lists per (query, k-range) under bf16
    # matmul scores (the last query block's second half arrives as two
    # 1024-wide lists); re-score the candidates exactly in fp32.
    # block layout: group g = qb*2+kh for g<15; g=15,16 are (qb7,kh1) pieces
    NC24 = 24
    cand = np.empty((B, L, NC24), np.int64)
    for ci in range(NCORES):
        b, s = divmod(ci, NSHARD)
        out = np.asarray(res.results[ci]["rarg"]).astype(np.int64)
        c = np.empty((128, NQB, NC24), np.int64)
        for qb in range(NQB):
            lists = []
            for kh in range(2):
                g = qb * 2 + kh
                if qb == NQB - 1 and kh == 1:
                    lists.append(np.clip(out[:, 120:128], 0, 1023) + 2048)
                    lists.append(np.clip(out[:, 128:136], 0, 1023) + 3072)
                else:
                    lists.append(np.clip(out[:, g * 8 : g * 8 + 8], 0, 2047) + kh * 2048)
            row = np.concatenate(lists, axis=1)  # [128, 16 or 24]
            if row.shape[1] < NC24:
                row = np.concatenate([row, row[:, : NC24 - row.shape[1]]], axis=1)
            c[:, qb, :] = row
        cand[b, s * LQ : (s + 1) * LQ] = c.transpose(1, 0, 2).reshape(LQ, NC24)

    rstar = np.empty((B, L), np.float32)
    rarg = np.empty((B, L), np.int64)
    for b in range(B):
        kc = Kn[b][:, cand[b].reshape(-1)].reshape(C9, L, NC24)
        scores = np.einsum("cqj,cq->qj", kc, Qn[b], dtype=np.float64)
        maxv = scores.max(axis=1, keepdims=True)
        kmask = np.where(scores == maxv, cand[b], 1 << 40)
        rarg[b] = kmask.min(axis=1)  # first occurrence on ties, like argmax
        rstar[b] = maxv[:, 0].astype(np.float32)

    V_unf = _unfold_torch(V)
    T_unf = np.take_along_axis(V_unf, rarg[:, None, :], axis=2)
    T = (_fold_torch(T_unf, H, W) / 9.0).astype(np.float32)
    S = rstar.reshape(B, 1, H, W)

    if _return_results:
        return (S, T), res
    return (S, T)
